# revision 48
# baseline (speedup 1.0000x reference)
"""FEDformer-style DecoderLayer on 8 trn2 NeuronCores (Bass/Tile).

Sharding: data-parallel over batch (B=16 -> 2/core); FourierBlock mode
weights [D,D,64] mode-sharded 8 ways with a bf16 AllToAll redistributing
per-mode spectra by batch. All FFTs are truncated DFT matmuls. Activations
fp16 (PE 1 cyc/row, DVE 2x), FEB weights/spectra bf16 (fp16-subnormal
scale), attention middle fp32/f32r.
"""
import sys
sys.path.insert(0, '/opt/trn_rl_repo')
import numpy as np
import ml_dtypes

import concourse.bass as bass
import concourse.bacc as bacc
import concourse.mybir as mybir
import concourse.tile as tile
from concourse.bass_utils import run_bass_kernel_spmd
from concourse.masks import make_identity

P = 128
B, L, S, D, H, M, DFF, CO = 16, 512, 1024, 512, 8, 64, 2048, 512
NC = 8
BL = B // NC            # 2 batches/core
MJ = M // NC            # 8 modes/core
DK = D // H             # 64
DT = D // P             # 4
LT = L // P             # 4
ST = S // P             # 8
FT = DFF // P           # 16
LP = L + 6              # padded length for avgpool halos

F32 = mybir.dt.float32
F32R = mybir.dt.float32r
BF16 = mybir.dt.bfloat16
FP16 = mybir.dt.float16
AF = mybir.ActivationFunctionType
OP = mybir.AluOpType
AX = mybir.AxisListType

_CACHE = {}


def _build():
    nc = bacc.Bacc("TRN2", target_bir_lowering=False, debug=False, num_devices=NC)

    def din(name, shape, dt=F32):
        return nc.dram_tensor(name, shape, dt, kind="ExternalInput")

    xall = din("xall", [B, LT, P, D], FP16)       # full x token-major
    xfm = din("xfm", [BL, DT, P, L], FP16)        # own x feature-major
    crs = din("crs", [BL, ST, P, D], FP16)        # cross token-major chunks
    fw512c = din("fw512c", [LT, P, 2 * MJ], FP16)  # per-core fwd DFT (m,r)
    fw512r = din("fw512r", [LT, P, P], FP16)
    fw1024r = din("fw1024r", [ST, P, P], FP16)
    iv512r = din("iv512r", [P, L], BF16)          # inverse DFT, A2A row order
    iv512b = din("iv512b", [P, L], F32R)          # inverse DFT, block rows
    febwr = din("febwr", [MJ, DT, P, D], BF16)
    febwi = din("febwi", [MJ, DT, P, D], BF16)
    wqT = din("wqT", [DT, P, D], FP16)
    wkT = din("wkT", [DT, P, D], FP16)
    wvT = din("wvT", [DT, P, D], FP16)
    woT = din("woT", [DT, P, D], FP16)
    dcb_kq = din("dcb_kq", [3, DT, P, 1])         # S*bk | L*bq | S*bv cols
    bo_pp = din("bo_pp", [DT, P, 1])
    wff1T = din("wff1T", [FT, DT, P, P], FP16)    # [ft][dc][p=d][ff-col]
    wff2T = din("wff2T", [FT, P, D], FP16)        # [fc][p=ff][e]
    wccT = din("wccT", [3, 3, DT, P, CO], FP16)   # [trend][shift][dc][p=k][co]
    gw1T = din("gw1T", [3, DT, P, D // 2], FP16)
    gb1 = din("gb1", [3, 2, P, 1])
    gw2T = din("gw2T", [3, 2, P, 4], FP16)        # col 3 zero-pad
    grow = din("grow", [1, 16])                   # gb2 x3 (4 each) | kinv(4)
    sign_r = din("sign_r", [P, 1])

    xout = nc.dram_tensor("xout", [BL, DT, P, L], FP16, kind="ExternalOutput")
    rtout = nc.dram_tensor("rtout", [BL, LT, P, CO], F32,
                           kind="ExternalOutput")

    cc_in = nc.dram_tensor("cc_in", [NC, BL, 4, MJ, D], F32)
    cc_out = nc.dram_tensor("cc_out", [NC, BL, 4, MJ, D], F32)

    ctxs = []

    with tile.TileContext(nc) as tc:
        def pool(name, bufs, space="SBUF"):
            cm = tc.tile_pool(name=name, bufs=bufs, space=space)
            p = cm.__enter__()
            ctxs.append(cm)
            return p

        cp = pool("cp", 1)
        act = pool("act", 1)
        xsp = pool("xsp", 2)         # rotating x-stage slots (padded fp16)
        wk1 = pool("wk1", 1)         # single-buffered transients
        wk2 = pool("wk2", 2)         # double-buffered streams
        psA = pool("psA", 4, "PSUM")
        psC = pool("psC", 2, "PSUM")
        psB = pool("psB", 2, "PSUM")

        # ---------------- constants / weights (loaded once) ----------------
        fwc_s = cp.tile([P, LT, 2 * MJ], FP16, tag="fwc")
        nc.sync.dma_start(fwc_s[:], fw512c.rearrange("c p m -> p c m"))
        xb_pre = []
        for pi in range(3):         # prefetch first token-major x chunks
            b, lh_i = divmod(pi, 2)
            xb = wk2.tile([P, 2, D], FP16, tag="xall_b", bufs=3)
            nc.sync.dma_start(
                xb[:], xall[b, 2 * lh_i:2 * lh_i + 2]
                .rearrange("t p d -> p t d"))
            xb_pre.append(xb)

        ident = cp.tile([P, P], F32, tag="ident")
        make_identity(nc, ident[:])
        warmid = psB.tile([P, P], F32, tag="psB")
        nc.tensor.transpose(warmid[:], ident[:], ident[:])
        fw512_s = cp.tile([P, LT, P], FP16, tag="fw512")
        nc.sync.dma_start(fw512_s[:], fw512r.rearrange("c p m -> p c m"))
        fw1024_s = cp.tile([P, ST, P], FP16, tag="fw1024")
        nc.sync.dma_start(fw1024_s[:], fw1024r.rearrange("c p m -> p c m"))
        iv512_s = cp.tile([P, L], BF16, tag="iv512")
        nc.sync.dma_start(iv512_s[:], iv512r[:])
        iv512b_s = cp.tile([P, L], F32R, tag="iv512b")
        nc.sync.dma_start(iv512b_s[:], iv512b[:])
        sign_s = cp.tile([P, 1], F32, tag="sign")
        nc.sync.dma_start(sign_s[:], sign_r[:])
        bo_s = cp.tile([P, DT, 1], F32, tag="bo")
        nc.sync.dma_start(bo_s[:], bo_pp.rearrange("c p o -> p c o"))
        gb1_s = cp.tile([P, 3, 2, 1], F32, tag="gb1")
        nc.sync.dma_start(gb1_s[:], gb1.rearrange("g h p o -> p g h o"))
        gw2_s = cp.tile([P, 3, 2, 4], FP16, tag="gw2")
        nc.sync.dma_start(gw2_s[:], gw2T.rearrange("g h p t -> p g h t"))
        dckq_s = cp.tile([P, 3, DT, 1], F32, tag="dckq")
        nc.sync.dma_start(dckq_s[:], dcb_kq.rearrange("k c p o -> p k c o"))
        grow_s = cp.tile([1, 16], F32, tag="grow")
        nc.sync.dma_start(grow_s[:], grow[:])
        gbc = cp.tile([P, 16], F32, tag="gbc")
        nc.gpsimd.partition_broadcast(gbc[:], grow_s[:])
        kinv_b = gbc[:, 12:15]




        # ============ Phase A1: FEB DFT (all batches, core's 8 modes) =======
        # out qft [d-part, dc, (b,16)] bf16; per b: psum [16,512] via
        # stationary fwc [128,16], moving xb [128,512]; then 4 fp32
        # transposes [16,128] -> [128,16].
        qft = act.tile([P, DT, B, 2 * MJ], BF16, tag="qft")
        for b in range(B):
            pq = psB.tile([2 * MJ, D], F32, tag="psB")
            for lh_i in range(2):
                pi = b * 2 + lh_i
                if pi < 3:
                    xb = xb_pre[pi]
                else:
                    xb = wk2.tile([P, 2, D], FP16, tag="xall_b", bufs=3)
                    nc.sync.dma_start(
                        xb[:], xall[b, 2 * lh_i:2 * lh_i + 2]
                        .rearrange("t p d -> p t d"))
                for lc in range(2):
                    gl = 2 * lh_i + lc
                    nc.tensor.matmul(pq[:], fwc_s[:, gl, :], xb[:, lc, :],
                                     start=(gl == 0), stop=(gl == LT - 1))
            qsb = wk1.tile([2 * MJ, D], F32, tag="qsb", bufs=2)
            nc.scalar.copy(qsb[:], pq[:])
            for dc in range(DT):
                pt = psB.tile([P, 2 * MJ], F32, tag="psB")
                nc.tensor.transpose(pt[:], qsb[:, dc * P:(dc + 1) * P],
                                    ident[0:2 * MJ, 0:2 * MJ])
                nc.scalar.copy(qft[:, dc, b, :], pt[:])

        # ============ Phase A2: mode matmuls -> cc_in pieces ================
        for j in range(MJ):
            wr_t = wk2.tile([P, DT, D], BF16, tag="febw")
            nc.scalar.dma_start(wr_t[:], febwr[j].rearrange("c p e -> p c e"))
            wi_t = wk2.tile([P, DT, D], BF16, tag="febw")
            nc.scalar.dma_start(wi_t[:], febwi[j].rearrange("c p e -> p c e"))
            g1 = psA.tile([32, 512], F32, tag="psA")
            g2 = psA.tile([32, 512], F32, tag="psA")
            for dc in range(DT):
                lh = qft[:, dc].rearrange("p b (r m) -> p (b r) m", r=2)[:, :, j]
                nc.tensor.matmul(g1[:], lh, wr_t[:, dc, :],
                                 start=(dc == 0), stop=(dc == DT - 1))
                nc.tensor.matmul(g2[:], lh, wi_t[:, dc, :],
                                 start=(dc == 0), stop=(dc == DT - 1))
            sg = wk1.tile([32, 2, 512], F32, tag="stg")
            nc.vector.tensor_copy(sg[:, 0, :], g1[:])
            nc.vector.tensor_copy(sg[:, 1, :], g2[:])
            ccv = cc_in.rearrange("n b f j d -> (n b) f j d")
            nc.sync.dma_start(ccv[:, 0:2, j, :], sg[:, 0, :])
            nc.sync.dma_start(ccv[:, 2:4, j, :], sg[:, 1, :])

        nc.gpsimd.collective_compute(
            "AllToAll", OP.bypass, replica_groups=[list(range(NC))],
            ins=[cc_in[:]], outs=[cc_out[:]])

        # bulk weights: scalar queue, behind the A2A-critical febw stream
        wq_s = cp.tile([P, DT, D], FP16, tag="wq")
        nc.scalar.dma_start(wq_s[:], wqT.rearrange("c p e -> p c e"))
        wk_s = cp.tile([P, DT, D], FP16, tag="wk")
        nc.scalar.dma_start(wk_s[:], wkT.rearrange("c p e -> p c e"))
        wv_s = cp.tile([P, DT, D], FP16, tag="wv")
        nc.scalar.dma_start(wv_s[:], wvT.rearrange("c p e -> p c e"))
        wo_s = cp.tile([P, DT, D], FP16, tag="wo")
        nc.scalar.dma_start(wo_s[:], woT.rearrange("c p e -> p c e"))
        gw1_s = cp.tile([P, 3, DT, D // 2], FP16, tag="gw1")
        nc.scalar.dma_start(gw1_s[:], gw1T.rearrange("g c p h -> p g c h"))
        w1_s = cp.tile([P, FT, DT, P], FP16, tag="w1")
        nc.scalar.dma_start(w1_s[:], wff1T.rearrange("f c p o -> p f c o"))
        w2_s = cp.tile([P, FT, D], FP16, tag="w2")
        nc.scalar.dma_start(w2_s[:], wff2T.rearrange("f p e -> p f e"))

        # ============ Phase C1: cross DFT + K/V proj (overlaps A2A) =========
        # crossFT [m,d] via stationary fw1024 chunks; then 4 fp32 transposes
        # per b -> crossF [d-part, m] fp16.
        vf_re = act.tile([DK, BL, D], F32, tag="vf_re")
        vf_im = act.tile([DK, BL, D], F32, tag="vf_im")
        kf_d = act.tile([P, BL, DT, P], FP16, tag="kf_d")
        qf_d = act.tile([P, BL, DT, P], FP16, tag="qf_d")
        crossF = act.tile([P, BL, DT, P], FP16, tag="crossF")

        for b in range(BL):
            pcf = psA.tile([P, 512], F32, tag="psA")
            for sc in range(ST):
                cx = wk2.tile([P, D], FP16, tag="crs_c")
                nc.sync.dma_start(cx[:], crs[b, sc])
                nc.tensor.matmul(pcf[:], fw1024_s[:, sc, :], cx[:],
                                 start=(sc == 0), stop=(sc == ST - 1))
            cft = wk1.tile([P, 512], F32, tag="cft")
            nc.scalar.copy(cft[:], pcf[:])
            for dc in range(DT):
                pt = psB.tile([P, P], F32, tag="psB")
                nc.tensor.transpose(pt[:], cft[:, dc * P:(dc + 1) * P],
                                    ident[:])
                nc.scalar.copy(crossF[:, b, dc, :], pt[:])
        for b in range(BL):
            for wmat, kq, dest in ((wk_s, 0, kf_d), (wv_s, 2, None)):
                vfd = wk1.tile([P, DT, P], F32, tag="vfd")
                for et in range(DT):
                    pk = psC.tile([P, P], F32, tag="psC")
                    for dc in range(DT):
                        nc.tensor.matmul(pk[:],
                                         wmat[:, dc, et * P:(et + 1) * P],
                                         crossF[:, b, dc, :],
                                         start=(dc == 0), stop=(dc == DT - 1))
                    if dest is not None:
                        tgt = dest[:, b, et, :]
                        nc.scalar.copy(tgt, pk[:])
                        nc.vector.tensor_add(tgt[:, 0:1], tgt[:, 0:1],
                                             dckq_s[:, kq, et, :])
                    else:
                        nc.scalar.copy(vfd[:, et, :], pk[:])
                        nc.vector.tensor_add(vfd[:, et, 0:1], vfd[:, et, 0:1],
                                             dckq_s[:, kq, et, :])
                if dest is None:
                    for et in range(DT):
                        pt = psB.tile([DK, P], F32, tag="psB")
                        nc.tensor.transpose(pt[:], vfd[:, et, 0:DK],
                                            ident[:])
                        nc.vector.tensor_copy(
                            vf_re[:, b, et * P:(et + 1) * P], pt[:])
                        pt2 = psB.tile([DK, P], F32, tag="psB")
                        nc.tensor.transpose(pt2[:], vfd[:, et, DK:P],
                                            ident[:])
                        nc.vector.tensor_copy(
                            vf_im[:, b, et * P:(et + 1) * P], pt2[:])

        # ============ Phase A4: A2A receive, IDFT, FEB residual -> x0 =======
        x0 = xsp.tile([P, BL, DT, LP], FP16, tag="xs")
        for b in range(BL):
            for dt_i in range(DT):
                nc.gpsimd.memset(x0[:, b, dt_i, 0:3], 0.0)
                nc.gpsimd.memset(x0[:, b, dt_i, L + 3:LP], 0.0)
                nc.sync.dma_start(x0[:, b, dt_i, 3:L + 3], xfm[b, dt_i])
            t1 = wk1.tile([P, D], F32, tag="a2a")
            t2 = wk1.tile([P, D], F32, tag="a2b")
            for n in range(NC):
                nc.sync.dma_start(t1[n * 16:n * 16 + 16], cc_out[n, b, 0:2])
                nc.sync.dma_start(t2[n * 16:n * 16 + 8], cc_out[n, b, 3])
                nc.sync.dma_start(t2[n * 16 + 8:n * 16 + 16], cc_out[n, b, 2])
            om = wk1.tile([P, D], BF16, tag="om")
            nc.vector.scalar_tensor_tensor(om[:], t2[:], sign_s[:], t1[:],
                                           op0=OP.mult, op1=OP.add)
            for et in range(DT):
                pi = psA.tile([P, 512], F32, tag="psA")
                nc.tensor.matmul(pi[:], om[:, et * P:(et + 1) * P], iv512_s[:],
                                 start=True, stop=True)
                nc.vector.tensor_add(x0[:, b, et, 3:L + 3],
                                     x0[:, b, et, 3:L + 3], pi[:])

        # ============ shared decomposition block ============================
        # xin/xout_t: [P, BL, DT, LP] fp16 padded; trend = G0*s3 + G1*a2 +
        # G2*a3 with prefix-summed gates (G2=g2/7, G1=g1/5+G2, G0=g0/3+G1).
        def load_wcc(widx):
            wcc_w = wk2.tile([P, 3, DT, CO], FP16, tag="wcc", bufs=1)
            nc.scalar.dma_start(wcc_w[:],
                                wccT[widx].rearrange("s c p o -> p s c o"))
            return wcc_w

        def gating(xin, widx, b):
                h = wk1.tile([P, 2, L], FP16, tag="g_h")
                for ht in range(2):
                    ph = psC.tile([P, 512], F32, tag="psC")
                    for dc in range(DT):
                        nc.tensor.matmul(ph[:],
                                         gw1_s[:, widx, dc, ht * P:(ht + 1) * P],
                                         xin[:, b, dc, 3:L + 3],
                                         start=(dc == 0), stop=(dc == DT - 1))
                    nc.scalar.activation(h[:, ht, :], ph[:], AF.Relu,
                                         bias=gb1_s[:, widx, ht, :], scale=1.0)
                gfm = wk1.tile([1, 3, L], FP16, tag="gfm")
                gb2_b = gbc[:, widx * 4:widx * 4 + 3]
                for lt_i in range(LT):
                    pg = psB.tile([P, 4], F32, tag="psB")
                    for hc in range(2):
                        nc.tensor.matmul(pg[:],
                                         h[:, hc, lt_i * P:(lt_i + 1) * P],
                                         gw2_s[:, widx, hc, :],
                                         start=(hc == 0), stop=(hc == 1))
                    gt = wk1.tile([P, 4], F32, tag="g_t")
                    nc.vector.tensor_add(gt[:, 0:3], pg[:, 0:3], gb2_b)
                    sm = wk1.tile([P, 1], F32, tag="g_sm")
                    nc.scalar.activation(gt[:, 0:3], gt[:, 0:3], AF.Exp,
                                         scale=1.0, accum_out=sm[:])
                    rc = wk1.tile([P, 1], F32, tag="g_rc")
                    nc.vector.reciprocal(rc[:], sm[:])
                    nc.scalar.mul(gt[:, 0:3], gt[:, 0:3], rc[:])
                    nc.vector.tensor_mul(gt[:, 0:3], gt[:, 0:3], kinv_b)
                    # prefix sums: col1 += col2; col0 += col1
                    nc.vector.tensor_add(gt[:, 1:2], gt[:, 1:2], gt[:, 2:3])
                    nc.vector.tensor_add(gt[:, 0:1], gt[:, 0:1], gt[:, 1:2])
                    pgt = psB.tile([1, 3, P], F32, tag="psB")
                    for e in range(3):
                        nc.tensor.transpose(pgt[:, e, :], gt[:, e:e + 1],
                                            ident[:])
                    nc.vector.tensor_copy(
                        gfm[:, :, lt_i * P:(lt_i + 1) * P], pgt[:])
                gbt = wk2.tile([P, 3, L], FP16, tag="g_gb")
                for e in range(3):
                    nc.gpsimd.partition_broadcast(gbt[:, e, :], gfm[:, e, :])
                return gbt

        def trendsub(xin, xout_t, widx, b, gbt, halo_out):
                trend_b = wk2.tile([P, DT, L + 2], FP16, tag="trend")
                xi = xin[:, b]                      # [P, DT, LP]
                tr = trend_b[:, :, 1:L + 1]         # [P, DT, L]
                g0 = gbt[:, 0:1, :].broadcast_to([P, DT, L])
                g1 = gbt[:, 1:2, :].broadcast_to([P, DT, L])
                g2 = gbt[:, 2:3, :].broadcast_to([P, DT, L])
                sx = wk1.tile([P, DT, L], FP16, tag="d_s")
                tmp = xout_t[:, b, :, 3:L + 3]     # scratch, overwritten last
                nc.vector.tensor_add(sx[:], xi[:, :, 2:L + 2],
                                     xi[:, :, 4:L + 4])
                nc.vector.tensor_add(sx[:], sx[:], xi[:, :, 3:L + 3])
                nc.vector.tensor_mul(tr, sx[:], g0)
                nc.vector.tensor_add(sx[:], xi[:, :, 1:L + 1],
                                     xi[:, :, 5:L + 5])
                nc.vector.tensor_mul(tmp, sx[:], g1)
                nc.vector.tensor_add(tr, tr, tmp)
                nc.vector.tensor_add(sx[:], xi[:, :, 0:L], xi[:, :, 6:LP])
                nc.vector.tensor_mul(tmp, sx[:], g2)
                nc.vector.tensor_add(tr, tr, tmp)
                if halo_out:
                    for dt_i in range(DT):
                        nc.gpsimd.memset(xout_t[:, b, dt_i, 0:3], 0.0)
                        nc.gpsimd.memset(xout_t[:, b, dt_i, L + 3:LP], 0.0)
                nc.vector.tensor_sub(xout_t[:, b, :, 3:L + 3],
                                     xi[:, :, 3:L + 3], tr)
                return trend_b

        def decomp(xin, xout_t, widx, halo_out):
            wcc_w = load_wcc(widx)
            gbts = [gating(xin, widx, b) for b in range(BL)]
            for b in range(BL):
                tb = trendsub(xin, xout_t, widx, b, gbts[b], halo_out)
                circpass_b(tb, wcc_w, widx, b)

        # ============ circ-conv partial pass (per batch) ====================
        def circpass_b(trend_b, wcc_w, widx, b):
            nc.gpsimd.tensor_copy(trend_b[:, :, 0:1], trend_b[:, :, L:L + 1])
            nc.gpsimd.tensor_copy(trend_b[:, :, L + 1:L + 2],
                                  trend_b[:, :, 1:2])
            for lt_i in range(LT):
                pr = psA.tile([P, 512], F32, tag="psA")
                first = True
                for s in range(3):
                    for dc in range(DT):
                        nc.tensor.matmul(
                            pr[:],
                            trend_b[:, dc, lt_i * P + s:lt_i * P + s + P],
                            wcc_w[:, s, dc, :],
                            start=first, stop=(s == 2 and dc == DT - 1))
                        first = False
                if widx == 0:
                    nc.scalar.copy(rt_acc[:, b, lt_i, :], pr[:])
                else:
                    nc.vector.tensor_add(rt_acc[:, b, lt_i, :],
                                         rt_acc[:, b, lt_i, :], pr[:])
                if widx == 2:
                    nc.sync.dma_start(rtout[b, lt_i], rt_acc[:, b, lt_i, :])

        rt_acc = act.tile([P, BL, LT, CO], F32, tag="rt_acc")

        x1 = xsp.tile([P, BL, DT, LP], FP16, tag="xs")
        decomp(x0, x1, 0, halo_out=False)

        # ============ Phase C2: Q proj + DFT ================================
        for b in range(BL):
            pqf = [psA.tile([P, P], F32, tag="psA", name=f"pqf{_i}")
                   for _i in range(DT)]
            for lc in range(LT):
                pk = psC.tile([P, 512], F32, tag="psC")
                for dc in range(DT):
                    nc.tensor.matmul(
                        pk[:], x1[:, b, dc, 3 + lc * P:3 + (lc + 1) * P],
                        wq_s[:, dc, :],
                        start=(dc == 0), stop=(dc == DT - 1))
                qt = wk2.tile([P, D], FP16, tag="kv_tt")
                nc.scalar.copy(qt[:], pk[:])
                for dt_i in range(DT):
                    nc.tensor.matmul(pqf[dt_i][:],
                                     qt[:, dt_i * P:(dt_i + 1) * P],
                                     fw512_s[:, lc, :],
                                     start=(lc == 0), stop=(lc == LT - 1),
                                     skip_group_check=True)
            for dt_i in range(DT):
                nc.scalar.copy(qf_d[:, b, dt_i, :], pqf[dt_i][:])
                nc.vector.tensor_add(qf_d[:, b, dt_i, 0:1],
                                     qf_d[:, b, dt_i, 0:1],
                                     dckq_s[:, 1, dt_i, :])

        # ============ attention =============================================
        of_sb = wk1.tile([P, BL, D], F32R, tag="of")
        for b in range(BL):
            sall = wk1.tile([DK, H, M], F32, tag="s_all")
            for hh in range(H):
                blk, half = hh // 2, (hh % 2) * DK
                pS = psB.tile([DK, M], F32, tag="psB")
                for ri in range(2):
                    nc.tensor.matmul(
                        pS[:],
                        qf_d[half:half + DK, b, blk, ri * M:(ri + 1) * M],
                        kf_d[half:half + DK, b, blk, ri * M:(ri + 1) * M],
                        start=(ri == 0), stop=(ri == 1))
                nc.vector.tensor_copy(sall[:, hh, :], pS[:])
            mx = wk1.tile([DK, H], F32, tag="s_mx")
            nc.vector.tensor_reduce(mx[:], sall[:], axis=AX.X, op=OP.max,
                                    negate=True)
            sm = wk1.tile([DK, H], F32, tag="s_sm")
            rc = wk1.tile([DK, H], F32, tag="s_rc")
            for hh in range(H):
                nc.scalar.activation(sall[:, hh, :], sall[:, hh, :], AF.Exp,
                                     bias=mx[:, hh:hh + 1], scale=1.0,
                                     accum_out=sm[:, hh:hh + 1])
            nc.vector.reciprocal(rc[:], sm[:])
            aT = wk1.tile([DK, H, M], F32, tag="a_T")
            for hh in range(H):
                nc.scalar.mul(sall[:, hh, :], sall[:, hh, :], rc[:, hh:hh + 1])
                pt = psB.tile([DK, M], F32, tag="psB")
                nc.tensor.transpose(pt[:], sall[:, hh, :], ident[0:DK, 0:DK])
                nc.vector.tensor_copy(aT[:, hh, :], pt[:])
            pof = psA.tile([P, 512], F32, tag="psA")
            for hh in range(H):
                nc.tensor.matmul(pof[0:DK, hh * DK:(hh + 1) * DK],
                                 aT[:, hh, :],
                                 vf_re[:, b, hh * DK:(hh + 1) * DK],
                                 start=True, stop=True)
                nc.tensor.matmul(pof[DK:P, hh * DK:(hh + 1) * DK],
                                 aT[:, hh, :],
                                 vf_im[:, b, hh * DK:(hh + 1) * DK],
                                 start=True, stop=True)
            nc.vector.tensor_copy(of_sb[:, b, :], pof[:])

        # idft (fm) -> wo proj + bias + residual -> x2
        x2 = xsp.tile([P, BL, DT, LP], FP16, tag="xs")
        for b in range(BL):
            for dt_i in range(DT):
                nc.gpsimd.memset(x2[:, b, dt_i, 0:3], 0.0)
                nc.gpsimd.memset(x2[:, b, dt_i, L + 3:LP], 0.0)
            apre = wk1.tile([P, DT, L], FP16, tag="apre")
            for et in range(DT):
                pi = psA.tile([P, 512], F32, tag="psA")
                nc.tensor.matmul(pi[:], of_sb[:, b, et * P:(et + 1) * P],
                                 iv512b_s[:], start=True, stop=True)
                nc.scalar.copy(apre[:, et, :], pi[:])
            for et in range(DT):
                po = psA.tile([P, 512], F32, tag="psA")
                for dc in range(DT):
                    nc.tensor.matmul(po[:], wo_s[:, dc, et * P:(et + 1) * P],
                                     apre[:, dc, :],
                                     start=(dc == 0), stop=(dc == DT - 1))
                nc.vector.scalar_tensor_tensor(
                    x2[:, b, et, 3:L + 3], po[:], bo_s[:, et, :],
                    x1[:, b, et, 3:L + 3], op0=OP.add, op1=OP.add)

        # ============ decomp2 / FFN / decomp3 ===============================
        x3 = xsp.tile([P, BL, DT, LP], FP16, tag="xs")
        decomp(x2, x3, 1, halo_out=False)

        # FFN interleaved per-batch with decomp3 gating+trend (circ deferred)
        x4 = xsp.tile([P, BL, DT, LP], FP16, tag="xs")
        x5 = xsp.tile([P, BL, DT, LP], FP16, tag="xs")
        wcc3 = load_wcc(2)
        trends3 = []
        for b in range(BL):
            for dt_i in range(DT):
                nc.gpsimd.memset(x4[:, b, dt_i, 0:3], 0.0)
                nc.gpsimd.memset(x4[:, b, dt_i, L + 3:LP], 0.0)
            hf = wk2.tile([P, FT, L], FP16, tag="ffn_h", bufs=1)
            for ft in range(FT):
                ph = psC.tile([P, 512], F32, tag="psC")
                for dc in range(DT):
                    nc.tensor.matmul(ph[:], w1_s[:, ft, dc, :],
                                     x3[:, b, dc, 3:L + 3],
                                     start=(dc == 0), stop=(dc == DT - 1))
                nc.scalar.activation(hf[:, ft, :], ph[:], AF.Relu)
            pys = [psA.tile([P, 512], F32, tag="psA", name=f"py{_i}")
                   for _i in range(DT)]
            for fc in range(FT):
                for et in range(DT):
                    nc.tensor.matmul(pys[et][:],
                                     w2_s[:, fc, et * P:(et + 1) * P],
                                     hf[:, fc, :],
                                     start=(fc == 0), stop=(fc == FT - 1))
            for et in range(DT):
                nc.vector.tensor_add(x4[:, b, et, 3:L + 3],
                                     x3[:, b, et, 3:L + 3], pys[et][:])
            gbt = gating(x4, 2, b)
            trends3.append(trendsub(x4, x5, 2, b, gbt, False))
        for b in range(BL):
            circpass_b(trends3[b], wcc3, 2, b)

        # ============ outputs (feature-major; host transposes) ==============
        for b in range(BL):
            for dt_i in range(DT):
                nc.sync.dma_start(xout[b, dt_i], x5[:, b, dt_i, 3:L + 3])

        for cm in reversed(ctxs):
            cm.__exit__(None, None, None)

    nc.compile()
    return nc


# ---------------------------------------------------------------------------
# host side
# ---------------------------------------------------------------------------
def _fwd_basis_cols(n, modes, interleave=False):
    l = np.arange(n)[:, None].astype(np.float64)
    m = np.asarray(modes)[None, :].astype(np.float64)
    th = 2.0 * np.pi * l * m / n
    cs, sn = np.cos(th), -np.sin(th)
    if interleave:
        out = np.empty((n, 2 * len(modes)))
        out[:, 0::2] = cs
        out[:, 1::2] = sn
        return out.astype(np.float32)
    return np.concatenate([cs, sn], axis=1).astype(np.float32)


def _inv_basis(n):
    l = np.arange(n)[None, :].astype(np.float64)
    m = np.arange(M)[:, None].astype(np.float64)
    c = np.where(np.arange(M) == 0, 1.0, 2.0)[:, None]
    th = 2.0 * np.pi * l * m / n
    return np.concatenate([c * np.cos(th) / n, -c * np.sin(th) / n],
                         axis=0).astype(np.float32)


def _prep_in_maps(x, cross, feb_wr, feb_wi, wq, bq, wk, bk, wv, bv, wo, bo,
                  w_ff1, w_ff2, d1_w1, d1_b1, d1_w2, d1_b2,
                  d2_w1, d2_b1, d2_w2, d2_b2, d3_w1, d3_b1, d3_w2, d3_b2,
                  p1, p2, p3):
    bf16 = ml_dtypes.bfloat16
    f16 = np.float16
    x = np.ascontiguousarray(x, np.float32)
    cross = np.ascontiguousarray(cross, np.float32)

    xall_np = np.ascontiguousarray(x.reshape(B, LT, P, D).astype(f16))
    xfm_full = np.ascontiguousarray(x.transpose(0, 2, 1)).reshape(B, DT, P, L) \
        .astype(f16)
    crs_full = np.ascontiguousarray(cross.reshape(B, ST, P, D)).astype(f16)

    fw512r_np = np.ascontiguousarray(
        _fwd_basis_cols(L, np.arange(M)).reshape(LT, P, P)).astype(f16)
    fw1024r_np = np.ascontiguousarray(
        _fwd_basis_cols(S, np.arange(M)).reshape(ST, P, P)).astype(f16)
    iv512b_np = _inv_basis(L)
    iv512_np = _inv_basis(L)
    # om rows arrive as (src_core n, ri, local mode j): row n*16+ri*8+j holds
    # (re if ri==0 else im) of global mode n*8+j
    perm = np.zeros(P, np.int64)
    for n_i in range(NC):
        for ri in range(2):
            for j_i in range(MJ):
                perm[n_i * 16 + ri * 8 + j_i] = ri * M + n_i * MJ + j_i
    iv512_np = np.ascontiguousarray(iv512_np[perm]).astype(bf16)

    wqT_np = np.ascontiguousarray(wq.T).reshape(DT, P, D).astype(f16)
    wkT_np = np.ascontiguousarray(wk.T).reshape(DT, P, D).astype(f16)
    wvT_np = np.ascontiguousarray(wv.T).reshape(DT, P, D).astype(f16)
    woT_np = np.ascontiguousarray(wo.T).reshape(DT, P, D).astype(f16)
    dcb_kq_np = np.stack([np.asarray(bk) * S, np.asarray(bq) * L,
                          np.asarray(bv) * S]) \
        .reshape(3, DT, P, 1).astype(np.float32)
    bo_np = np.ascontiguousarray(bo).reshape(DT, P, 1).astype(np.float32)
    wff1_np = np.ascontiguousarray(
        w_ff1.T.reshape(DT, P, FT, P).transpose(2, 0, 1, 3)).astype(f16)
    wff2_np = np.ascontiguousarray(w_ff2.T).reshape(FT, P, D).astype(f16)
    wcc_np = np.zeros((3, 3, DT, P, CO), f16)
    for w_i, p_i in enumerate((p1, p2, p3)):
        for s in range(3):
            wcc_np[w_i, s] = np.ascontiguousarray(p_i[:, :, s].T) \
                .reshape(DT, P, CO).astype(f16)
    gw1_np = np.stack([np.ascontiguousarray(w.T).reshape(DT, P, D // 2)
                       for w in (d1_w1, d2_w1, d3_w1)]).astype(f16)
    gb1_np = np.stack([np.asarray(b_).reshape(2, P, 1)
                       for b_ in (d1_b1, d2_b1, d3_b1)]).astype(np.float32)
    gw2_np = np.zeros((3, 2, P, 4), f16)
    for i, w in enumerate((d1_w2, d2_w2, d3_w2)):
        gw2_np[i, :, :, 0:3] = np.ascontiguousarray(w.T).reshape(2, P, 3) \
            .astype(f16)
    grow_np = np.zeros((1, 16), np.float32)
    for i, b2 in enumerate((d1_b2, d2_b2, d3_b2)):
        grow_np[0, i * 4:i * 4 + 3] = np.asarray(b2, np.float32)
    grow_np[0, 12:15] = [1.0 / 3.0, 1.0 / 5.0, 1.0 / 7.0]
    sign_np = np.tile(np.concatenate([-np.ones(8), np.ones(8)]), NC) \
        .reshape(P, 1).astype(np.float32)

    in_maps = []
    for c in range(NC):
        bs = slice(BL * c, BL * (c + 1))
        modes = np.arange(MJ * c, MJ * (c + 1))
        in_maps.append(dict(
            xall=xall_np,
            xfm=xfm_full[bs],
            crs=crs_full[bs],
            fw512c=np.ascontiguousarray(
                _fwd_basis_cols(L, modes).astype(f16)
                .reshape(LT, P, 2 * MJ)),
            fw512r=fw512r_np, fw1024r=fw1024r_np, iv512r=iv512_np,
            iv512b=iv512b_np,
            febwr=np.ascontiguousarray(
                feb_wr[:, :, modes].transpose(2, 0, 1)).astype(bf16)
                .reshape(MJ, DT, P, D),
            febwi=np.ascontiguousarray(
                feb_wi[:, :, modes].transpose(2, 0, 1)).astype(bf16)
                .reshape(MJ, DT, P, D),
            wqT=wqT_np, wkT=wkT_np, wvT=wvT_np, woT=woT_np,
            dcb_kq=dcb_kq_np, bo_pp=bo_np,
            wff1T=wff1_np, wff2T=wff2_np, wccT=wcc_np,
            gw1T=gw1_np, gb1=gb1_np, gw2T=gw2_np,
            grow=grow_np, sign_r=sign_np,
        ))

    return in_maps


def kernel(**inputs):
    if "nc" not in _CACHE:
        _CACHE["nc"] = _build()
    nc = _CACHE["nc"]
    in_maps = _prep_in_maps(**inputs)
    _CACHE["in_maps"] = in_maps
    res = run_bass_kernel_spmd(nc, in_maps, list(range(NC)))
    xo = np.zeros((B, L, D), np.float32)
    rt = np.zeros((B, L, CO), np.float32)
    for c in range(NC):
        r = res.results[c]
        xo[BL * c:BL * (c + 1)] = np.asarray(r["xout"]).astype(np.float32) \
            .reshape(BL, D, L).transpose(0, 2, 1)
        rt[BL * c:BL * (c + 1)] = np.asarray(r["rtout"]).astype(np.float32) \
            .reshape(BL, L, CO)
    return xo, rt


# revision 49
# speedup vs baseline: 1.0435x; 1.0435x over previous
"""FEDformer-style DecoderLayer on 8 trn2 NeuronCores (Bass/Tile).

Sharding: data-parallel over batch (B=16 -> 2/core); FourierBlock mode
weights [D,D,64] mode-sharded 8 ways with a bf16 AllToAll redistributing
per-mode spectra by batch. All FFTs are truncated DFT matmuls. Activations
fp16 (PE 1 cyc/row, DVE 2x), FEB weights/spectra bf16 (fp16-subnormal
scale), attention middle fp32/f32r.
"""
import sys
sys.path.insert(0, '/opt/trn_rl_repo')
import numpy as np
import ml_dtypes

import concourse.bass as bass
import concourse.bacc as bacc
import concourse.mybir as mybir
import concourse.tile as tile
from concourse.bass_utils import run_bass_kernel_spmd
from concourse.masks import make_identity

P = 128
B, L, S, D, H, M, DFF, CO = 16, 512, 1024, 512, 8, 64, 2048, 512
NC = 8
BL = B // NC            # 2 batches/core
MJ = M // NC            # 8 modes/core
DK = D // H             # 64
DT = D // P             # 4
LT = L // P             # 4
ST = S // P             # 8
FT = DFF // P           # 16
LP = L + 6              # padded length for avgpool halos

F32 = mybir.dt.float32
F32R = mybir.dt.float32r
BF16 = mybir.dt.bfloat16
FP16 = mybir.dt.float16
AF = mybir.ActivationFunctionType
OP = mybir.AluOpType
AX = mybir.AxisListType

_CACHE = {}


def _build():
    nc = bacc.Bacc("TRN2", target_bir_lowering=False, debug=False, num_devices=NC)

    def din(name, shape, dt=F32):
        return nc.dram_tensor(name, shape, dt, kind="ExternalInput")

    xall = din("xall", [B, LT, P, D], FP16)       # full x token-major
    xfm = din("xfm", [BL, DT, P, L], FP16)        # own x feature-major
    crs = din("crs", [BL, ST, P, D], FP16)        # cross token-major chunks
    fw512c = din("fw512c", [LT, P, 2 * MJ], FP16)  # per-core fwd DFT (m,r)
    fw512r = din("fw512r", [LT, P, P], FP16)
    fw1024r = din("fw1024r", [ST, P, P], FP16)
    iv512r = din("iv512r", [P, L], BF16)          # inverse DFT, A2A row order
    iv512b = din("iv512b", [P, L], F32R)          # inverse DFT, block rows
    febwr = din("febwr", [MJ, DT, P, D], BF16)
    febwi = din("febwi", [MJ, DT, P, D], BF16)
    wqT = din("wqT", [DT, P, D], FP16)
    wkT = din("wkT", [DT, P, D], FP16)
    wvT = din("wvT", [DT, P, D], FP16)
    woT = din("woT", [DT, P, D], FP16)
    dcb_kq = din("dcb_kq", [3, DT, P, 1])         # S*bk | L*bq | S*bv cols
    bo_pp = din("bo_pp", [DT, P, 1])
    wff1T = din("wff1T", [FT, DT, P, P], FP16)    # [ft][dc][p=d][ff-col]
    wff2T = din("wff2T", [FT, P, D], FP16)        # [fc][p=ff][e]
    wccT = din("wccT", [3, 3, DT, P, CO], FP16)   # [trend][shift][dc][p=k][co]
    gw1T = din("gw1T", [3, DT, P, D // 2], FP16)
    gb1 = din("gb1", [3, 2, P, 1])
    gw2T = din("gw2T", [3, 2, P, 4], FP16)        # col 3 zero-pad
    grow = din("grow", [1, 16])                   # gb2 x3 (4 each) | kinv(4)
    sign_r = din("sign_r", [P, 1])

    xout = nc.dram_tensor("xout", [BL, DT, P, L], FP16, kind="ExternalOutput")
    rtout = nc.dram_tensor("rtout", [BL, LT, P, CO], F32,
                           kind="ExternalOutput")

    cc_in = nc.dram_tensor("cc_in", [NC, BL, 4, MJ, D], F32)
    cc_out = nc.dram_tensor("cc_out", [NC, BL, 4, MJ, D], F32)

    ctxs = []

    with tile.TileContext(nc) as tc:
        def pool(name, bufs, space="SBUF"):
            cm = tc.tile_pool(name=name, bufs=bufs, space=space)
            p = cm.__enter__()
            ctxs.append(cm)
            return p

        cp = pool("cp", 1)
        act = pool("act", 1)
        xsp = pool("xsp", 2)         # rotating x-stage slots (padded fp16)
        wk1 = pool("wk1", 1)         # single-buffered transients
        wk2 = pool("wk2", 2)         # double-buffered streams
        psA = pool("psA", 4, "PSUM")
        psC = pool("psC", 2, "PSUM")
        psB = pool("psB", 2, "PSUM")

        # ---------------- constants / weights (loaded once) ----------------
        fwc_s = cp.tile([P, LT, 2 * MJ], FP16, tag="fwc")
        nc.sync.dma_start(fwc_s[:], fw512c.rearrange("c p m -> p c m"))
        xb_pre = []
        for pi in range(3):         # prefetch first token-major x chunks
            b, lh_i = divmod(pi, 2)
            xb = wk2.tile([P, 2, D], FP16, tag="xall_b", bufs=3)
            nc.sync.dma_start(
                xb[:], xall[b, 2 * lh_i:2 * lh_i + 2]
                .rearrange("t p d -> p t d"))
            xb_pre.append(xb)

        ident = cp.tile([P, P], F32, tag="ident")
        make_identity(nc, ident[:])
        warmid = psB.tile([P, P], F32, tag="psB")
        nc.tensor.transpose(warmid[:], ident[:], ident[:])
        fw512_s = cp.tile([P, LT, P], FP16, tag="fw512")
        nc.sync.dma_start(fw512_s[:], fw512r.rearrange("c p m -> p c m"))
        fw1024_s = cp.tile([P, ST, P], FP16, tag="fw1024")
        nc.sync.dma_start(fw1024_s[:], fw1024r.rearrange("c p m -> p c m"))
        iv512_s = cp.tile([P, L], BF16, tag="iv512")
        nc.sync.dma_start(iv512_s[:], iv512r[:])
        iv512b_s = cp.tile([P, L], F32R, tag="iv512b")
        nc.sync.dma_start(iv512b_s[:], iv512b[:])
        sign_s = cp.tile([P, 1], F32, tag="sign")
        nc.sync.dma_start(sign_s[:], sign_r[:])
        bo_s = cp.tile([P, DT, 1], F32, tag="bo")
        nc.sync.dma_start(bo_s[:], bo_pp.rearrange("c p o -> p c o"))
        gb1_s = cp.tile([P, 3, 2, 1], F32, tag="gb1")
        nc.sync.dma_start(gb1_s[:], gb1.rearrange("g h p o -> p g h o"))
        gw2_s = cp.tile([P, 3, 2, 4], FP16, tag="gw2")
        nc.sync.dma_start(gw2_s[:], gw2T.rearrange("g h p t -> p g h t"))
        dckq_s = cp.tile([P, 3, DT, 1], F32, tag="dckq")
        nc.sync.dma_start(dckq_s[:], dcb_kq.rearrange("k c p o -> p k c o"))
        grow_s = cp.tile([1, 16], F32, tag="grow")
        nc.sync.dma_start(grow_s[:], grow[:])
        gbc = cp.tile([P, 16], F32, tag="gbc")
        nc.gpsimd.partition_broadcast(gbc[:], grow_s[:])
        kinv_b = gbc[:, 12:15]




        # ============ Phase A1: FEB DFT (all batches, core's 8 modes) =======
        # out qft [d-part, dc, (b,16)] bf16; per b: psum [16,512] via
        # stationary fwc [128,16], moving xb [128,512]; then 4 fp32
        # transposes [16,128] -> [128,16].
        qft = act.tile([P, DT, B, 2 * MJ], BF16, tag="qft")
        for b in range(B):
            pq = psB.tile([2 * MJ, D], F32, tag="psB")
            for lh_i in range(2):
                pi = b * 2 + lh_i
                if pi < 3:
                    xb = xb_pre[pi]
                else:
                    xb = wk2.tile([P, 2, D], FP16, tag="xall_b", bufs=3)
                    nc.sync.dma_start(
                        xb[:], xall[b, 2 * lh_i:2 * lh_i + 2]
                        .rearrange("t p d -> p t d"))
                for lc in range(2):
                    gl = 2 * lh_i + lc
                    nc.tensor.matmul(pq[:], fwc_s[:, gl, :], xb[:, lc, :],
                                     start=(gl == 0), stop=(gl == LT - 1))
            qsb = wk1.tile([2 * MJ, D], F32, tag="qsb", bufs=2)
            nc.scalar.copy(qsb[:], pq[:])
            for dc in range(DT):
                pt = psB.tile([P, 2 * MJ], F32, tag="psB")
                nc.tensor.transpose(pt[:], qsb[:, dc * P:(dc + 1) * P],
                                    ident[0:2 * MJ, 0:2 * MJ])
                nc.vector.tensor_copy(qft[:, dc, b, :], pt[:])

        # ============ Phase A2: mode matmuls -> cc_in pieces ================
        for j in range(MJ):
            wr_t = wk2.tile([P, DT, D], BF16, tag="febw")
            nc.scalar.dma_start(wr_t[:], febwr[j].rearrange("c p e -> p c e"))
            wi_t = wk2.tile([P, DT, D], BF16, tag="febw")
            nc.scalar.dma_start(wi_t[:], febwi[j].rearrange("c p e -> p c e"))
            g1 = psA.tile([32, 512], F32, tag="psA")
            g2 = psA.tile([32, 512], F32, tag="psA")
            for dc in range(DT):
                lh = qft[:, dc].rearrange("p b (r m) -> p (b r) m", r=2)[:, :, j]
                nc.tensor.matmul(g1[:], lh, wr_t[:, dc, :],
                                 start=(dc == 0), stop=(dc == DT - 1))
                nc.tensor.matmul(g2[:], lh, wi_t[:, dc, :],
                                 start=(dc == 0), stop=(dc == DT - 1))
            sg = wk1.tile([32, 2, 512], F32, tag="stg")
            nc.vector.tensor_copy(sg[:, 0, :], g1[:])
            nc.vector.tensor_copy(sg[:, 1, :], g2[:])
            ccv = cc_in.rearrange("n b f j d -> (n b) f j d")
            nc.sync.dma_start(ccv[:, 0:2, j, :], sg[:, 0, :])
            nc.sync.dma_start(ccv[:, 2:4, j, :], sg[:, 1, :])

        nc.gpsimd.collective_compute(
            "AllToAll", OP.bypass, replica_groups=[list(range(NC))],
            ins=[cc_in[:]], outs=[cc_out[:]])

        # bulk weights: scalar queue, behind the A2A-critical febw stream
        wq_s = cp.tile([P, DT, D], FP16, tag="wq")
        nc.scalar.dma_start(wq_s[:], wqT.rearrange("c p e -> p c e"))
        wk_s = cp.tile([P, DT, D], FP16, tag="wk")
        nc.scalar.dma_start(wk_s[:], wkT.rearrange("c p e -> p c e"))
        wv_s = cp.tile([P, DT, D], FP16, tag="wv")
        nc.scalar.dma_start(wv_s[:], wvT.rearrange("c p e -> p c e"))
        wo_s = cp.tile([P, DT, D], FP16, tag="wo")
        nc.scalar.dma_start(wo_s[:], woT.rearrange("c p e -> p c e"))
        gw1_s = cp.tile([P, 3, DT, D // 2], FP16, tag="gw1")
        nc.scalar.dma_start(gw1_s[:], gw1T.rearrange("g c p h -> p g c h"))
        w1_s = cp.tile([P, FT, DT, P], FP16, tag="w1")
        nc.scalar.dma_start(w1_s[:], wff1T.rearrange("f c p o -> p f c o"))
        w2_s = cp.tile([P, FT, D], FP16, tag="w2")
        nc.scalar.dma_start(w2_s[:], wff2T.rearrange("f p e -> p f e"))

        # ============ Phase C1: cross DFT + K/V proj (overlaps A2A) =========
        # crossFT [m,d] via stationary fw1024 chunks; then 4 fp32 transposes
        # per b -> crossF [d-part, m] fp16.
        vf_re = act.tile([DK, BL, D], F32, tag="vf_re")
        vf_im = act.tile([DK, BL, D], F32, tag="vf_im")
        kf_d = act.tile([P, BL, DT, P], FP16, tag="kf_d")
        qf_d = act.tile([P, BL, DT, P], FP16, tag="qf_d")
        crossF = act.tile([P, BL, DT, P], FP16, tag="crossF")

        for b in range(BL):
            pcf = psA.tile([P, 512], F32, tag="psA")
            for sc in range(ST):
                cx = wk2.tile([P, D], FP16, tag="crs_c")
                nc.sync.dma_start(cx[:], crs[b, sc])
                nc.tensor.matmul(pcf[:], fw1024_s[:, sc, :], cx[:],
                                 start=(sc == 0), stop=(sc == ST - 1))
            cft = wk1.tile([P, 512], F32, tag="cft")
            nc.scalar.copy(cft[:], pcf[:])
            for dc in range(DT):
                pt = psB.tile([P, P], F32, tag="psB")
                nc.tensor.transpose(pt[:], cft[:, dc * P:(dc + 1) * P],
                                    ident[:])
                nc.scalar.copy(crossF[:, b, dc, :], pt[:])
        for b in range(BL):
            for wmat, kq, dest in ((wk_s, 0, kf_d), (wv_s, 2, None)):
                vfd = wk1.tile([P, DT, P], F32, tag="vfd")
                for et in range(DT):
                    pk = psC.tile([P, P], F32, tag="psC")
                    for dc in range(DT):
                        nc.tensor.matmul(pk[:],
                                         wmat[:, dc, et * P:(et + 1) * P],
                                         crossF[:, b, dc, :],
                                         start=(dc == 0), stop=(dc == DT - 1))
                    if dest is not None:
                        tgt = dest[:, b, et, :]
                        nc.scalar.copy(tgt, pk[:])
                        nc.vector.tensor_add(tgt[:, 0:1], tgt[:, 0:1],
                                             dckq_s[:, kq, et, :])
                    else:
                        nc.scalar.copy(vfd[:, et, :], pk[:])
                        nc.vector.tensor_add(vfd[:, et, 0:1], vfd[:, et, 0:1],
                                             dckq_s[:, kq, et, :])
                if dest is None:
                    for et in range(DT):
                        pt = psB.tile([DK, P], F32, tag="psB")
                        nc.tensor.transpose(pt[:], vfd[:, et, 0:DK],
                                            ident[:])
                        nc.vector.tensor_copy(
                            vf_re[:, b, et * P:(et + 1) * P], pt[:])
                        pt2 = psB.tile([DK, P], F32, tag="psB")
                        nc.tensor.transpose(pt2[:], vfd[:, et, DK:P],
                                            ident[:])
                        nc.vector.tensor_copy(
                            vf_im[:, b, et * P:(et + 1) * P], pt2[:])

        # ============ Phase A4: A2A receive, IDFT, FEB residual -> x0 =======
        x0 = xsp.tile([P, BL, DT, LP], FP16, tag="xs")
        for b in range(BL):
            for dt_i in range(DT):
                nc.gpsimd.memset(x0[:, b, dt_i, 0:3], 0.0)
                nc.gpsimd.memset(x0[:, b, dt_i, L + 3:LP], 0.0)
                nc.sync.dma_start(x0[:, b, dt_i, 3:L + 3], xfm[b, dt_i])
            t1 = wk1.tile([P, D], F32, tag="a2a")
            t2 = wk1.tile([P, D], F32, tag="a2b")
            for n in range(NC):
                nc.sync.dma_start(t1[n * 16:n * 16 + 16], cc_out[n, b, 0:2])
                nc.sync.dma_start(t2[n * 16:n * 16 + 8], cc_out[n, b, 3])
                nc.sync.dma_start(t2[n * 16 + 8:n * 16 + 16], cc_out[n, b, 2])
            om = wk1.tile([P, D], BF16, tag="om")
            nc.vector.scalar_tensor_tensor(om[:], t2[:], sign_s[:], t1[:],
                                           op0=OP.mult, op1=OP.add)
            for et in range(DT):
                pi = psA.tile([P, 512], F32, tag="psA")
                nc.tensor.matmul(pi[:], om[:, et * P:(et + 1) * P], iv512_s[:],
                                 start=True, stop=True)
                nc.vector.tensor_add(x0[:, b, et, 3:L + 3],
                                     x0[:, b, et, 3:L + 3], pi[:])

        # ============ shared decomposition block ============================
        # xin/xout_t: [P, BL, DT, LP] fp16 padded; trend = G0*s3 + G1*a2 +
        # G2*a3 with prefix-summed gates (G2=g2/7, G1=g1/5+G2, G0=g0/3+G1).
        def load_wcc(widx):
            wcc_w = wk2.tile([P, 3, DT, CO], FP16, tag="wcc", bufs=1)
            nc.scalar.dma_start(wcc_w[:],
                                wccT[widx].rearrange("s c p o -> p s c o"))
            return wcc_w

        def gating(xin, widx, b):
                h = wk1.tile([P, 2, L], FP16, tag="g_h")
                for ht in range(2):
                    ph = psC.tile([P, 512], F32, tag="psC")
                    for dc in range(DT):
                        nc.tensor.matmul(ph[:],
                                         gw1_s[:, widx, dc, ht * P:(ht + 1) * P],
                                         xin[:, b, dc, 3:L + 3],
                                         start=(dc == 0), stop=(dc == DT - 1))
                    nc.scalar.activation(h[:, ht, :], ph[:], AF.Relu,
                                         bias=gb1_s[:, widx, ht, :], scale=1.0)
                gfm = wk1.tile([1, 3, L], FP16, tag="gfm")
                gb2_b = gbc[:, widx * 4:widx * 4 + 3]
                for lt_i in range(LT):
                    pg = psB.tile([P, 4], F32, tag="psB")
                    for hc in range(2):
                        nc.tensor.matmul(pg[:],
                                         h[:, hc, lt_i * P:(lt_i + 1) * P],
                                         gw2_s[:, widx, hc, :],
                                         start=(hc == 0), stop=(hc == 1))
                    gt = wk1.tile([P, 4], F32, tag="g_t")
                    nc.vector.tensor_add(gt[:, 0:3], pg[:, 0:3], gb2_b)
                    sm = wk1.tile([P, 1], F32, tag="g_sm")
                    nc.scalar.activation(gt[:, 0:3], gt[:, 0:3], AF.Exp,
                                         scale=1.0, accum_out=sm[:])
                    rc = wk1.tile([P, 1], F32, tag="g_rc")
                    nc.vector.reciprocal(rc[:], sm[:])
                    nc.scalar.mul(gt[:, 0:3], gt[:, 0:3], rc[:])
                    nc.vector.tensor_mul(gt[:, 0:3], gt[:, 0:3], kinv_b)
                    # prefix sums: col1 += col2; col0 += col1
                    nc.vector.tensor_add(gt[:, 1:2], gt[:, 1:2], gt[:, 2:3])
                    nc.vector.tensor_add(gt[:, 0:1], gt[:, 0:1], gt[:, 1:2])
                    pgt = psB.tile([1, 3, P], F32, tag="psB")
                    for e in range(3):
                        nc.tensor.transpose(pgt[:, e, :], gt[:, e:e + 1],
                                            ident[:])
                    nc.vector.tensor_copy(
                        gfm[:, :, lt_i * P:(lt_i + 1) * P], pgt[:])
                gbt = wk2.tile([P, 3, L], FP16, tag="g_gb")
                for e in range(3):
                    nc.gpsimd.partition_broadcast(gbt[:, e, :], gfm[:, e, :])
                return gbt

        def trendsub(xin, xout_t, widx, b, gbt, halo_out):
                trend_b = wk2.tile([P, DT, L + 2], FP16, tag="trend")
                xi = xin[:, b]                      # [P, DT, LP]
                tr = trend_b[:, :, 1:L + 1]         # [P, DT, L]
                g0 = gbt[:, 0:1, :].broadcast_to([P, DT, L])
                g1 = gbt[:, 1:2, :].broadcast_to([P, DT, L])
                g2 = gbt[:, 2:3, :].broadcast_to([P, DT, L])
                sx = wk1.tile([P, DT, L], FP16, tag="d_s")
                tmp = xout_t[:, b, :, 3:L + 3]     # scratch, overwritten last
                nc.vector.tensor_add(sx[:], xi[:, :, 2:L + 2],
                                     xi[:, :, 4:L + 4])
                nc.vector.tensor_add(sx[:], sx[:], xi[:, :, 3:L + 3])
                nc.vector.tensor_mul(tr, sx[:], g0)
                nc.vector.tensor_add(sx[:], xi[:, :, 1:L + 1],
                                     xi[:, :, 5:L + 5])
                nc.vector.tensor_mul(tmp, sx[:], g1)
                nc.vector.tensor_add(tr, tr, tmp)
                nc.vector.tensor_add(sx[:], xi[:, :, 0:L], xi[:, :, 6:LP])
                nc.vector.tensor_mul(tmp, sx[:], g2)
                nc.vector.tensor_add(tr, tr, tmp)
                if halo_out:
                    for dt_i in range(DT):
                        nc.gpsimd.memset(xout_t[:, b, dt_i, 0:3], 0.0)
                        nc.gpsimd.memset(xout_t[:, b, dt_i, L + 3:LP], 0.0)
                nc.vector.tensor_sub(xout_t[:, b, :, 3:L + 3],
                                     xi[:, :, 3:L + 3], tr)
                return trend_b

        def decomp(xin, xout_t, widx, halo_out):
            wcc_w = load_wcc(widx)
            gbts = [gating(xin, widx, b) for b in range(BL)]
            for b in range(BL):
                tb = trendsub(xin, xout_t, widx, b, gbts[b], halo_out)
                circpass_b(tb, wcc_w, widx, b)

        # ============ circ-conv partial pass (per batch) ====================
        def circpass_b(trend_b, wcc_w, widx, b):
            nc.gpsimd.tensor_copy(trend_b[:, :, 0:1], trend_b[:, :, L:L + 1])
            nc.gpsimd.tensor_copy(trend_b[:, :, L + 1:L + 2],
                                  trend_b[:, :, 1:2])
            for lt_i in range(LT):
                pr = psA.tile([P, 512], F32, tag="psA")
                first = True
                for s in range(3):
                    for dc in range(DT):
                        nc.tensor.matmul(
                            pr[:],
                            trend_b[:, dc, lt_i * P + s:lt_i * P + s + P],
                            wcc_w[:, s, dc, :],
                            start=first, stop=(s == 2 and dc == DT - 1))
                        first = False
                if widx == 0:
                    nc.scalar.copy(rt_acc[:, b, lt_i, :], pr[:])
                else:
                    nc.vector.tensor_add(rt_acc[:, b, lt_i, :],
                                         rt_acc[:, b, lt_i, :], pr[:])
                if widx == 2:
                    nc.sync.dma_start(rtout[b, lt_i], rt_acc[:, b, lt_i, :])

        rt_acc = act.tile([P, BL, LT, CO], F32, tag="rt_acc")

        x1 = xsp.tile([P, BL, DT, LP], FP16, tag="xs")
        decomp(x0, x1, 0, halo_out=False)

        # ============ Phase C2: Q proj + DFT ================================
        for b in range(BL):
            pqf = [psA.tile([P, P], F32, tag="psA", name=f"pqf{_i}")
                   for _i in range(DT)]
            for lc in range(LT):
                pk = psC.tile([P, 512], F32, tag="psC")
                for dc in range(DT):
                    nc.tensor.matmul(
                        pk[:], x1[:, b, dc, 3 + lc * P:3 + (lc + 1) * P],
                        wq_s[:, dc, :],
                        start=(dc == 0), stop=(dc == DT - 1))
                qt = wk2.tile([P, D], FP16, tag="kv_tt")
                nc.scalar.copy(qt[:], pk[:])
                for dt_i in range(DT):
                    nc.tensor.matmul(pqf[dt_i][:],
                                     qt[:, dt_i * P:(dt_i + 1) * P],
                                     fw512_s[:, lc, :],
                                     start=(lc == 0), stop=(lc == LT - 1),
                                     skip_group_check=True)
            for dt_i in range(DT):
                nc.scalar.copy(qf_d[:, b, dt_i, :], pqf[dt_i][:])
                nc.vector.tensor_add(qf_d[:, b, dt_i, 0:1],
                                     qf_d[:, b, dt_i, 0:1],
                                     dckq_s[:, 1, dt_i, :])

        # ============ attention =============================================
        of_sb = wk1.tile([P, BL, D], F32R, tag="of")
        for b in range(BL):
            sall = wk1.tile([DK, H, M], F32, tag="s_all")
            for hh in range(H):
                blk, half = hh // 2, (hh % 2) * DK
                pS = psB.tile([DK, M], F32, tag="psB")
                for ri in range(2):
                    nc.tensor.matmul(
                        pS[:],
                        qf_d[half:half + DK, b, blk, ri * M:(ri + 1) * M],
                        kf_d[half:half + DK, b, blk, ri * M:(ri + 1) * M],
                        start=(ri == 0), stop=(ri == 1))
                nc.vector.tensor_copy(sall[:, hh, :], pS[:])
            mx = wk1.tile([DK, H], F32, tag="s_mx")
            nc.vector.tensor_reduce(mx[:], sall[:], axis=AX.X, op=OP.max,
                                    negate=True)
            sm = wk1.tile([DK, H], F32, tag="s_sm")
            rc = wk1.tile([DK, H], F32, tag="s_rc")
            for hh in range(H):
                nc.scalar.activation(sall[:, hh, :], sall[:, hh, :], AF.Exp,
                                     bias=mx[:, hh:hh + 1], scale=1.0,
                                     accum_out=sm[:, hh:hh + 1])
            nc.vector.reciprocal(rc[:], sm[:])
            aT = wk1.tile([DK, H, M], F32, tag="a_T")
            for hh in range(H):
                nc.scalar.mul(sall[:, hh, :], sall[:, hh, :], rc[:, hh:hh + 1])
                pt = psB.tile([DK, M], F32, tag="psB")
                nc.tensor.transpose(pt[:], sall[:, hh, :], ident[0:DK, 0:DK])
                nc.vector.tensor_copy(aT[:, hh, :], pt[:])
            pof = psA.tile([P, 512], F32, tag="psA")
            for hh in range(H):
                nc.tensor.matmul(pof[0:DK, hh * DK:(hh + 1) * DK],
                                 aT[:, hh, :],
                                 vf_re[:, b, hh * DK:(hh + 1) * DK],
                                 start=True, stop=True)
                nc.tensor.matmul(pof[DK:P, hh * DK:(hh + 1) * DK],
                                 aT[:, hh, :],
                                 vf_im[:, b, hh * DK:(hh + 1) * DK],
                                 start=True, stop=True)
            nc.vector.tensor_copy(of_sb[:, b, :], pof[:])

        # idft (fm) -> wo proj + bias + residual -> x2
        x2 = xsp.tile([P, BL, DT, LP], FP16, tag="xs")
        for b in range(BL):
            for dt_i in range(DT):
                nc.gpsimd.memset(x2[:, b, dt_i, 0:3], 0.0)
                nc.gpsimd.memset(x2[:, b, dt_i, L + 3:LP], 0.0)
            apre = wk1.tile([P, DT, L], FP16, tag="apre")
            for et in range(DT):
                pi = psA.tile([P, 512], F32, tag="psA")
                nc.tensor.matmul(pi[:], of_sb[:, b, et * P:(et + 1) * P],
                                 iv512b_s[:], start=True, stop=True)
                nc.scalar.copy(apre[:, et, :], pi[:])
            for et in range(DT):
                po = psA.tile([P, 512], F32, tag="psA")
                for dc in range(DT):
                    nc.tensor.matmul(po[:], wo_s[:, dc, et * P:(et + 1) * P],
                                     apre[:, dc, :],
                                     start=(dc == 0), stop=(dc == DT - 1))
                nc.vector.scalar_tensor_tensor(
                    x2[:, b, et, 3:L + 3], po[:], bo_s[:, et, :],
                    x1[:, b, et, 3:L + 3], op0=OP.add, op1=OP.add)

        # ============ decomp2 / FFN / decomp3 ===============================
        x3 = xsp.tile([P, BL, DT, LP], FP16, tag="xs")
        decomp(x2, x3, 1, halo_out=False)

        # FFN interleaved per-batch with decomp3 gating+trend (circ deferred)
        x4 = xsp.tile([P, BL, DT, LP], FP16, tag="xs")
        x5 = xsp.tile([P, BL, DT, LP], FP16, tag="xs")
        wcc3 = load_wcc(2)
        trends3 = []
        for b in range(BL):
            for dt_i in range(DT):
                nc.gpsimd.memset(x4[:, b, dt_i, 0:3], 0.0)
                nc.gpsimd.memset(x4[:, b, dt_i, L + 3:LP], 0.0)
            hf = wk2.tile([P, FT, L], FP16, tag="ffn_h", bufs=1)
            for ft in range(FT):
                ph = psC.tile([P, 512], F32, tag="psC")
                for dc in range(DT):
                    nc.tensor.matmul(ph[:], w1_s[:, ft, dc, :],
                                     x3[:, b, dc, 3:L + 3],
                                     start=(dc == 0), stop=(dc == DT - 1))
                nc.scalar.activation(hf[:, ft, :], ph[:], AF.Relu)
            pys = [psA.tile([P, 512], F32, tag="psA", name=f"py{_i}")
                   for _i in range(DT)]
            for fc in range(FT):
                for et in range(DT):
                    nc.tensor.matmul(pys[et][:],
                                     w2_s[:, fc, et * P:(et + 1) * P],
                                     hf[:, fc, :],
                                     start=(fc == 0), stop=(fc == FT - 1))
            for et in range(DT):
                nc.vector.tensor_add(x4[:, b, et, 3:L + 3],
                                     x3[:, b, et, 3:L + 3], pys[et][:])
            gbt = gating(x4, 2, b)
            trends3.append(trendsub(x4, x5, 2, b, gbt, False))
        for b in range(BL):
            circpass_b(trends3[b], wcc3, 2, b)

        # ============ outputs (feature-major; host transposes) ==============
        for b in range(BL):
            for dt_i in range(DT):
                nc.sync.dma_start(xout[b, dt_i], x5[:, b, dt_i, 3:L + 3])

        for cm in reversed(ctxs):
            cm.__exit__(None, None, None)

    nc.compile()
    return nc


# ---------------------------------------------------------------------------
# host side
# ---------------------------------------------------------------------------
def _fwd_basis_cols(n, modes, interleave=False):
    l = np.arange(n)[:, None].astype(np.float64)
    m = np.asarray(modes)[None, :].astype(np.float64)
    th = 2.0 * np.pi * l * m / n
    cs, sn = np.cos(th), -np.sin(th)
    if interleave:
        out = np.empty((n, 2 * len(modes)))
        out[:, 0::2] = cs
        out[:, 1::2] = sn
        return out.astype(np.float32)
    return np.concatenate([cs, sn], axis=1).astype(np.float32)


def _inv_basis(n):
    l = np.arange(n)[None, :].astype(np.float64)
    m = np.arange(M)[:, None].astype(np.float64)
    c = np.where(np.arange(M) == 0, 1.0, 2.0)[:, None]
    th = 2.0 * np.pi * l * m / n
    return np.concatenate([c * np.cos(th) / n, -c * np.sin(th) / n],
                         axis=0).astype(np.float32)


def _prep_in_maps(x, cross, feb_wr, feb_wi, wq, bq, wk, bk, wv, bv, wo, bo,
                  w_ff1, w_ff2, d1_w1, d1_b1, d1_w2, d1_b2,
                  d2_w1, d2_b1, d2_w2, d2_b2, d3_w1, d3_b1, d3_w2, d3_b2,
                  p1, p2, p3):
    bf16 = ml_dtypes.bfloat16
    f16 = np.float16
    x = np.ascontiguousarray(x, np.float32)
    cross = np.ascontiguousarray(cross, np.float32)

    xall_np = np.ascontiguousarray(x.reshape(B, LT, P, D).astype(f16))
    xfm_full = np.ascontiguousarray(x.transpose(0, 2, 1)).reshape(B, DT, P, L) \
        .astype(f16)
    crs_full = np.ascontiguousarray(cross.reshape(B, ST, P, D)).astype(f16)

    fw512r_np = np.ascontiguousarray(
        _fwd_basis_cols(L, np.arange(M)).reshape(LT, P, P)).astype(f16)
    fw1024r_np = np.ascontiguousarray(
        _fwd_basis_cols(S, np.arange(M)).reshape(ST, P, P)).astype(f16)
    iv512b_np = _inv_basis(L)
    iv512_np = _inv_basis(L)
    # om rows arrive as (src_core n, ri, local mode j): row n*16+ri*8+j holds
    # (re if ri==0 else im) of global mode n*8+j
    perm = np.zeros(P, np.int64)
    for n_i in range(NC):
        for ri in range(2):
            for j_i in range(MJ):
                perm[n_i * 16 + ri * 8 + j_i] = ri * M + n_i * MJ + j_i
    iv512_np = np.ascontiguousarray(iv512_np[perm]).astype(bf16)

    wqT_np = np.ascontiguousarray(wq.T).reshape(DT, P, D).astype(f16)
    wkT_np = np.ascontiguousarray(wk.T).reshape(DT, P, D).astype(f16)
    wvT_np = np.ascontiguousarray(wv.T).reshape(DT, P, D).astype(f16)
    woT_np = np.ascontiguousarray(wo.T).reshape(DT, P, D).astype(f16)
    dcb_kq_np = np.stack([np.asarray(bk) * S, np.asarray(bq) * L,
                          np.asarray(bv) * S]) \
        .reshape(3, DT, P, 1).astype(np.float32)
    bo_np = np.ascontiguousarray(bo).reshape(DT, P, 1).astype(np.float32)
    wff1_np = np.ascontiguousarray(
        w_ff1.T.reshape(DT, P, FT, P).transpose(2, 0, 1, 3)).astype(f16)
    wff2_np = np.ascontiguousarray(w_ff2.T).reshape(FT, P, D).astype(f16)
    wcc_np = np.zeros((3, 3, DT, P, CO), f16)
    for w_i, p_i in enumerate((p1, p2, p3)):
        for s in range(3):
            wcc_np[w_i, s] = np.ascontiguousarray(p_i[:, :, s].T) \
                .reshape(DT, P, CO).astype(f16)
    gw1_np = np.stack([np.ascontiguousarray(w.T).reshape(DT, P, D // 2)
                       for w in (d1_w1, d2_w1, d3_w1)]).astype(f16)
    gb1_np = np.stack([np.asarray(b_).reshape(2, P, 1)
                       for b_ in (d1_b1, d2_b1, d3_b1)]).astype(np.float32)
    gw2_np = np.zeros((3, 2, P, 4), f16)
    for i, w in enumerate((d1_w2, d2_w2, d3_w2)):
        gw2_np[i, :, :, 0:3] = np.ascontiguousarray(w.T).reshape(2, P, 3) \
            .astype(f16)
    grow_np = np.zeros((1, 16), np.float32)
    for i, b2 in enumerate((d1_b2, d2_b2, d3_b2)):
        grow_np[0, i * 4:i * 4 + 3] = np.asarray(b2, np.float32)
    grow_np[0, 12:15] = [1.0 / 3.0, 1.0 / 5.0, 1.0 / 7.0]
    sign_np = np.tile(np.concatenate([-np.ones(8), np.ones(8)]), NC) \
        .reshape(P, 1).astype(np.float32)

    in_maps = []
    for c in range(NC):
        bs = slice(BL * c, BL * (c + 1))
        modes = np.arange(MJ * c, MJ * (c + 1))
        in_maps.append(dict(
            xall=xall_np,
            xfm=xfm_full[bs],
            crs=crs_full[bs],
            fw512c=np.ascontiguousarray(
                _fwd_basis_cols(L, modes).astype(f16)
                .reshape(LT, P, 2 * MJ)),
            fw512r=fw512r_np, fw1024r=fw1024r_np, iv512r=iv512_np,
            iv512b=iv512b_np,
            febwr=np.ascontiguousarray(
                feb_wr[:, :, modes].transpose(2, 0, 1)).astype(bf16)
                .reshape(MJ, DT, P, D),
            febwi=np.ascontiguousarray(
                feb_wi[:, :, modes].transpose(2, 0, 1)).astype(bf16)
                .reshape(MJ, DT, P, D),
            wqT=wqT_np, wkT=wkT_np, wvT=wvT_np, woT=woT_np,
            dcb_kq=dcb_kq_np, bo_pp=bo_np,
            wff1T=wff1_np, wff2T=wff2_np, wccT=wcc_np,
            gw1T=gw1_np, gb1=gb1_np, gw2T=gw2_np,
            grow=grow_np, sign_r=sign_np,
        ))

    return in_maps


def kernel(**inputs):
    if "nc" not in _CACHE:
        _CACHE["nc"] = _build()
    nc = _CACHE["nc"]
    in_maps = _prep_in_maps(**inputs)
    _CACHE["in_maps"] = in_maps
    res = run_bass_kernel_spmd(nc, in_maps, list(range(NC)))
    xo = np.zeros((B, L, D), np.float32)
    rt = np.zeros((B, L, CO), np.float32)
    for c in range(NC):
        r = res.results[c]
        xo[BL * c:BL * (c + 1)] = np.asarray(r["xout"]).astype(np.float32) \
            .reshape(BL, D, L).transpose(0, 2, 1)
        rt[BL * c:BL * (c + 1)] = np.asarray(r["rtout"]).astype(np.float32) \
            .reshape(BL, L, CO)
    return xo, rt
